# revision 2
# baseline (speedup 1.0000x reference)
"""Trainium2 Bass kernel for nn_LowRankSVDBlock (8-core SPMD).

Sharding: data-parallel over batch (2 groups of 4 cores); within a group,
tensor-parallel over heads for attention (4 heads/core) and token-parallel
(512 tokens/core) for the out-projection second stage + FFN, with one
ReduceScatter after the out-projection first stage.

Layout convention on device: activations are kept TRANSPOSED (features on
partitions, tokens on the free dim) so every matmul uses natural-layout
weights as the stationary operand. Attention scores are computed transposed
(kv on partitions, q on free) so softmax normalization sums come from a
ones-column appended to V, and P^T feeds the PV matmul directly.
"""

import numpy as np
import ml_dtypes
from contextlib import ExitStack

import concourse.bass as bass
import concourse.tile as tile
from concourse import bacc, mybir
from concourse import bass_utils

BF16 = mybir.dt.bfloat16
F32 = mybir.dt.float32
AF = mybir.ActivationFunctionType
ALU = mybir.AluOpType

B, S, D, H, DH = 2, 2048, 1024, 16, 64
R = 32          # attention rank
ROUT = 512      # out-proj rank
I = 4096        # ffn inner
RFC = 512       # fc rank
NCORE = 8
TOK = 512       # tokens per core in FFN phase
HPC = 4         # heads per core
LN_EPS = 1e-5

_cache = {}


def _build_program(single_core=False, phases=99):
    nc = bacc.Bacc("TRN2", target_bir_lowering=False, debug=False,
                   num_devices=1 if single_core else NCORE)

    def din(name, shape, dt):
        return nc.dram_tensor(name, list(shape), dt, kind="ExternalInput")

    hbT = din("hbT", (D, S), BF16)            # hidden[b].T (bf16)
    hrT = din("hrT", (D, TOK), F32)           # hidden rows for this core, T
    ln1g = din("ln1g", (128, 8), F32)
    ln1b = din("ln1b", (128, 8), F32)
    ln2g = din("ln2g", (128, 8), F32)
    ln2b = din("ln2b", (128, 8), F32)
    uq = din("uq", (D, HPC * R), BF16)
    uk = din("uk", (D, HPC * R), BF16)
    uv = din("uv", (D, HPC * R), BF16)
    vq = din("vq", (R, HPC * DH), BF16)
    vk = din("vk", (R, HPC * DH), BF16)
    vv = din("vv", (R, HPC * DH), BF16)
    bq = din("bq", (DH, HPC), F32)
    bk = din("bk", (DH, HPC), F32)
    bv = din("bv", (DH, HPC), F32)
    uqc = din("uqc", (128, 1), F32)
    ukc = din("ukc", (128, 1), F32)
    uvc = din("uvc", (128, 1), F32)
    uqb = din("uqb", (128, 1), F32)
    ukb = din("ukb", (128, 1), F32)
    uvb = din("uvb", (128, 1), F32)
    tri = din("tri", (128, 128), BF16)        # triu: allow kv<=q
    ones1 = din("ones1", (128, 1), BF16)
    ouT = din("ouT", (2 * 128, ROUT), BF16)   # out_U rows for this core
    ovT = din("ovT", (ROUT, D), BF16)
    ob = din("ob", (128, 8), F32)
    f1u = din("f1u", (D, RFC), BF16)
    f1v = din("f1v", (RFC, I), BF16)
    f1b = din("f1b", (128, 32), F32)
    f2u = din("f2u", (I, RFC), BF16)
    f2v = din("f2v", (RFC, D), BF16)
    f2b = din("f2b", (128, 8), F32)
    out_t = nc.dram_tensor("out_t", [D, TOK], F32, kind="ExternalOutput")

    with tile.TileContext(nc) as tc, ExitStack() as top:
        # ---- persistent pools (small constants + attention weights)
        wp = top.enter_context(tc.tile_pool(name="weights", bufs=1))
        cp = top.enter_context(tc.tile_pool(name="consts", bufs=1))

        def wtile(pool, shape, dt, tag):
            return pool.tile(list(shape), dt, name=tag)

        # constants / biases
        tri_t = cp.tile([128, 128], BF16, name="tri")
        nc.sync.dma_start(tri_t[:], tri[:, :])
        ones_t = cp.tile([128, 1], BF16, name="ones1")
        nc.sync.dma_start(ones_t[:], ones1[:, :])
        small_consts = [("ln1g", ln1g), ("ln1b", ln1b), ("ln2g", ln2g),
                        ("ln2b", ln2b), ("ob", ob), ("f2b", f2b)]
        csb = {}
        for nm, t in small_consts:
            csb[nm] = cp.tile([128, 8], F32, name=nm)
            nc.sync.dma_start(csb[nm][:], t[:, :])
        f1b_sb = cp.tile([128, 32], F32, name="f1b")
        nc.sync.dma_start(f1b_sb[:], f1b[:, :])
        bqkv_sb = []
        for nm, t in [("bq", bq), ("bk", bk), ("bv", bv)]:
            bt = cp.tile([DH, HPC], F32, name=nm)
            nc.sync.dma_start(bt[:], t[:, :])
            bqkv_sb.append(bt)
        ucs_sb, uub_sb = [], []
        for nm, t in [("uqc", uqc), ("ukc", ukc), ("uvc", uvc)]:
            bt = cp.tile([128, 1], F32, name=nm)
            nc.sync.dma_start(bt[:], t[:, :])
            ucs_sb.append(bt)
        for nm, t in [("uqb", uqb), ("ukb", ukb), ("uvb", uvb)]:
            bt = cp.tile([128, 1], F32, name=nm)
            nc.sync.dma_start(bt[:], t[:, :])
            uub_sb.append(bt)
        zero128 = cp.tile([128, 1], F32, name="zero128")
        nc.vector.memset(zero128[:], 0.0)
        eps1 = cp.tile([1, 1], F32, name="eps1")
        nc.vector.memset(eps1[:], LN_EPS)

        # attention weights
        u_sb = []
        for nm, t in [("uq", uq), ("uk", uk), ("uv", uv)]:
            w = wtile(wp, (128, 8 * 128), BF16, nm)
            for fc in range(8):
                nc.sync.dma_start(w[:, fc * 128:(fc + 1) * 128],
                                  t[fc * 128:(fc + 1) * 128, :])
            u_sb.append(w)
        v2_sb = []
        for nm, t in [("vq", vq), ("vk", vk), ("vv", vv)]:
            w = wtile(wp, (R, HPC * DH), BF16, nm)
            nc.sync.dma_start(w[:], t[:, :])
            v2_sb.append(w)
        ouT_sb = wtile(wp, (128, 2 * ROUT), BF16, "ouT")
        for c in range(2):
            nc.sync.dma_start(ouT_sb[:, c * ROUT:(c + 1) * ROUT],
                              ouT[c * 128:(c + 1) * 128, :])

        # pools spanning attention (freed before FFN weights load)
        attn_stack = top.enter_context(ExitStack())
        qk_pool = attn_stack.enter_context(tc.tile_pool(name="qkattn", bufs=1))
        yn_pool = attn_stack.enter_context(tc.tile_pool(name="yn", bufs=1))
        QT = [qk_pool.tile([DH, S], BF16, name=f"QT{h}") for h in range(HPC)]
        KT = [qk_pool.tile([DH, S], BF16, name=f"KT{h}") for h in range(HPC)]
        VA = [qk_pool.tile([128, 16 * (DH + 1)], BF16, name=f"VA{h}")
              for h in range(HPC)]
        YnT = [yn_pool.tile([128, S], BF16, name=f"Yn{f}") for f in range(2)]

        with ExitStack() as phABC:
            hbp = phABC.enter_context(tc.tile_pool(name="hb", bufs=1))
            bcp = phABC.enter_context(tc.tile_pool(name="bc", bufs=1))
            hb = [hbp.tile([128, S], BF16, name=f"hb{fc}")
                  for fc in range(8)]
            for fc in range(8):
                nc.sync.dma_start(hb[fc][:],
                                  hbT[fc * 128:(fc + 1) * 128, :])
            r_bs = [bcp.tile([128, 512], F32, name=f"rb{t}") for t in range(4)]
            s1_bs = [bcp.tile([128, 512], F32, name=f"sb{t}")
                     for t in range(4)]

            # ------------- Phase A: LN1 stats only -------------
            with ExitStack() as ph:
                tmp = ph.enter_context(tc.tile_pool(name="lntmp", bufs=6))
                stp = ph.enter_context(tc.tile_pool(name="lnstat", bufs=3))
                sps = ph.enter_context(
                    tc.tile_pool(name="lnps", bufs=2, space="PSUM"))
                for tck in range(4):
                    sl = slice(tck * 512, (tck + 1) * 512)
                    sum_ps = sps.tile([1, 512], F32, name="sum")
                    sq_ps = sps.tile([1, 512], F32, name="sq")
                    for fc in range(8):
                        sq = tmp.tile([128, 512], BF16, name="sqt")
                        nc.vector.tensor_mul(sq[:], hb[fc][:, sl],
                                             hb[fc][:, sl])
                        nc.tensor.matmul(sum_ps[:], ones_t[:], hb[fc][:, sl],
                                         start=(fc == 0), stop=(fc == 7))
                        nc.tensor.matmul(sq_ps[:], ones_t[:], sq[:],
                                         start=(fc == 0), stop=(fc == 7))
                    mu_c = stp.tile([1, 512], F32, name="mu_c")
                    w1 = stp.tile([1, 512], F32, name="w1")
                    t2 = stp.tile([1, 512], F32, name="t2")
                    nc.scalar.mul(mu_c[:], sum_ps[:], 1.0 / D)
                    nc.scalar.mul(w1[:], sq_ps[:], 1.0 / D)
                    nc.vector.tensor_mul(t2[:], mu_c[:], mu_c[:])
                    nc.vector.tensor_sub(w1[:], w1[:], t2[:])
                    nc.scalar.activation(t2[:], w1[:], AF.Sqrt, bias=eps1[:])
                    nc.vector.reciprocal(w1[:], t2[:])
                    nc.vector.tensor_mul(t2[:], mu_c[:], w1[:])
                    nc.gpsimd.partition_broadcast(r_bs[tck][:], w1[:])
                    nc.gpsimd.partition_broadcast(s1_bs[tck][:], t2[:])

            # ------------- Phase B+C: QKV projections -------------
            with ExitStack() as ph:
                rkp = ph.enter_context(tc.tile_pool(name="rk", bufs=1))
                qps = ph.enter_context(
                    tc.tile_pool(name="qkvps", bufs=1, space="PSUM"))
                s2ps = ph.enter_context(
                    tc.tile_pool(name="s2ps", bufs=2, space="PSUM"))
                rk = [[rkp.tile([R, S], BF16, name=f"rk{p}{h}")
                       for h in range(HPC)] for p in range(3)]
                ctp = ph.enter_context(tc.tile_pool(name="corr", bufs=4))
                for p in range(3):
                    pss = [qps.tile([128, 512], F32, name=f"qk{t}")
                           for t in range(4)]
                    for fc in range(8):
                        for tcix in range(4):
                            sl = slice(tcix * 512, (tcix + 1) * 512)
                            nc.tensor.matmul(
                                pss[tcix][:],
                                u_sb[p][:, fc * 128:(fc + 1) * 128],
                                hb[fc][:, sl],
                                start=(fc == 0), stop=(fc == 7))
                    for tcix in range(4):
                        sl = slice(tcix * 512, (tcix + 1) * 512)
                        corr = ctp.tile([128, 512], F32, name="corr")
                        nc.vector.tensor_mul(corr[:], pss[tcix][:],
                                             r_bs[tcix][:])
                        nc.vector.scalar_tensor_tensor(
                            corr[:], s1_bs[tcix][:], ucs_sb[p][:],
                            corr[:], ALU.mult, ALU.add)
                        for h in range(HPC):
                            nc.vector.tensor_scalar(
                                rk[p][h][:, sl],
                                corr[h * R:(h + 1) * R, :],
                                uub_sb[p][h * R:(h + 1) * R, :], None,
                                ALU.add)
                # stage 2: Q^T, K^T per head (+bias, per-partition)
                for p, dest in ((0, QT), (1, KT)):
                    for h in range(HPC):
                        for tcix in range(4):
                            sl = slice(tcix * 512, (tcix + 1) * 512)
                            ps = s2ps.tile([DH, 512], F32, name="s2")
                            nc.tensor.matmul(
                                ps[:], v2_sb[p][:, h * DH:(h + 1) * DH],
                                rk[p][h][:, sl], start=True, stop=True)
                            nc.vector.tensor_scalar(
                                dest[h][:, sl], ps[:],
                                bqkv_sb[p][:, h:h + 1], None, ALU.add)
                # stage 2: V natural [kv, dh] (+ones col; bias folded in Yn)
                for h in range(HPC):
                    nc.vector.memset(VA[h][:], 1.0)
                    for kt in range(16):
                        ps = s2ps.tile([128, DH], F32, name="s2v")
                        nc.tensor.matmul(
                            ps[:], rk[2][h][:, kt * 128:(kt + 1) * 128],
                            v2_sb[2][:, h * DH:(h + 1) * DH],
                            start=True, stop=True)
                        nc.vector.tensor_copy(
                            VA[h][:, kt * 65:kt * 65 + DH], ps[:])

        # ---------------- Phase D: attention ----------------
        if phases < 2:
            return nc, None
        with ExitStack() as ph:
            scps = ph.enter_context(
                tc.tile_pool(name="scps", bufs=2, space="PSUM"))
            scp2 = ph.enter_context(
                tc.tile_pool(name="scp2", bufs=2, space="PSUM"))
            pvps = ph.enter_context(
                tc.tile_pool(name="pvps", bufs=2, space="PSUM"))
            ptp = ph.enter_context(tc.tile_pool(name="pt", bufs=6))
            nrm = ph.enter_context(tc.tile_pool(name="nrm", bufs=3))
            for h in range(HPC):
                for qc in range(4):
                    q0 = qc * 512
                    ntile = 4 * qc + 4
                    pv = pvps.tile([DH + 1, 512], F32, name="pv")
                    t = 0
                    while t < ntile:
                        p = t - 4 * qc
                        if p < 0 and t + 1 < 4 * qc:
                            # two full kv tiles: batched exp over 2 banks
                            s2 = scp2.tile([128, 1024], F32, name="s2p")
                            for j in range(2):
                                nc.tensor.matmul(
                                    s2[:, j * 512:(j + 1) * 512],
                                    KT[h][:, (t + j) * 128:(t + j + 1) * 128],
                                    QT[h][:, q0:q0 + 512],
                                    start=True, stop=True)
                            pt2 = ptp.tile([128, 1024], BF16, name="p")
                            nc.scalar.activation(pt2[:], s2[:], AF.Exp,
                                                 bias=zero128[:], scale=0.125)
                            for j in range(2):
                                nc.tensor.matmul(
                                    pv[:], VA[h][:, (t + j) * 65:
                                                 (t + j + 1) * 65],
                                    pt2[:, j * 512:(j + 1) * 512],
                                    start=(t + j == 0), stop=False)
                            t += 2
                            continue
                        c0 = 128 * p if p >= 0 else 0
                        s_ps = scps.tile([128, 512], F32, name="s")
                        nc.tensor.matmul(
                            s_ps[:, c0:], KT[h][:, t * 128:(t + 1) * 128],
                            QT[h][:, q0 + c0:q0 + 512],
                            start=True, stop=True)
                        pt = ptp.tile([128, 512], BF16, name="p")
                        nc.scalar.activation(pt[:, c0:], s_ps[:, c0:],
                                             AF.Exp, bias=zero128[:],
                                             scale=0.125)
                        if p >= 0:
                            nc.vector.tensor_mul(pt[:, c0:c0 + 128],
                                                 pt[:, c0:c0 + 128],
                                                 tri_t[:])
                        nc.tensor.matmul(
                            pv[:, c0:], VA[h][:, t * 65:(t + 1) * 65],
                            pt[:, c0:], start=(t == 0),
                            stop=(t == ntile - 1))
                        t += 1
                    rec = nrm.tile([1, 512], F32, name="rec")
                    nc.vector.reciprocal(rec[:], pv[DH:DH + 1, :])
                    recb = nrm.tile([DH, 512], F32, name="recb")
                    nc.gpsimd.partition_broadcast(recb[:], rec[:])
                    ysl = YnT[h // 2][(h % 2) * DH:(h % 2 + 1) * DH,
                                      q0:q0 + 512]
                    yt = nrm.tile([DH, 512], F32, name="yt")
                    nc.vector.tensor_mul(yt[:], pv[0:DH, :], recb[:])
                    nc.vector.tensor_scalar(ysl, yt[:],
                                            bqkv_sb[2][:, h:h + 1], None,
                                            ALU.add)

        # -------- Phase E: out-proj stage 1 -> DRAM; ReduceScatter --------
        if phases < 3:
            return nc, None
        dramp = top.enter_context(tc.tile_pool(name="dram", bufs=1,
                                               space="DRAM"))
        rs_in = dramp.tile([4 * ROUT, 512], F32, name="rs_in")
        rs_out = dramp.tile([ROUT, 512], F32, name="rs_out")
        with ExitStack() as ph:
            fps = ph.enter_context(
                tc.tile_pool(name="o1ps", bufs=3, space="PSUM"))
            stg = ph.enter_context(tc.tile_pool(name="o1st", bufs=4))
            for qc in range(4):
                for rt in range(4):
                    ps = fps.tile([128, 512], F32, name="o1")
                    for f in range(2):
                        nc.tensor.matmul(
                            ps[:],
                            ouT_sb[:, f * ROUT + rt * 128:
                                   f * ROUT + (rt + 1) * 128],
                            YnT[f][:, qc * 512:(qc + 1) * 512],
                            start=(f == 0), stop=(f == 1))
                    st = stg.tile([128, 512], F32, name="st")
                    nc.vector.tensor_copy(st[:], ps[:])
                    nc.sync.dma_start(
                        rs_in[qc * 512 + rt * 128:
                              qc * 512 + (rt + 1) * 128, :], st[:])
            if single_core:
                nc.sync.dma_start(rs_out[:, :], rs_in[0:ROUT, :])
            else:
                nc.gpsimd.collective_compute(
                    "ReduceScatter", ALU.add,
                    replica_groups=[[0, 1, 2, 3], [4, 5, 6, 7]],
                    ins=[rs_in.opt()], outs=[rs_out.opt()])

        attn_stack.close()
        if phases < 4:
            return nc, None

        # -------- late weights (loaded while attention runs down) --------
        wp2 = top.enter_context(tc.tile_pool(name="weights2", bufs=1))

        def load_chunked(nm, t, rows, cols, dt=BF16):
            nchunk = rows // 128
            w = wtile(wp2, (128, nchunk * cols), dt, nm)
            for c in range(nchunk):
                nc.sync.dma_start(w[:, c * cols:(c + 1) * cols],
                                  t[c * 128:(c + 1) * 128, :])
            return w

        ovT_sb = load_chunked("ovT", ovT, ROUT, D)
        f1u_sb = load_chunked("f1u", f1u, D, RFC)
        f1v_sb = load_chunked("f1v", f1v, RFC, I)
        f2u_sb = load_chunked("f2u", f2u, I, RFC)
        f2v_sb = load_chunked("f2v", f2v, RFC, D)
        hr_sb = [wtile(wp2, (128, TOK), F32, f"hr{ft}") for ft in range(8)]
        for ft in range(8):
            nc.sync.dma_start(hr_sb[ft][:], hrT[ft * 128:(ft + 1) * 128, :])

        # -------- Phase G/H: out-proj stage 2 + residual -> h^T --------
        hp = top.enter_context(tc.tile_pool(name="hT", bufs=1))
        hT = [hp.tile([128, TOK], F32, name=f"h{ft}") for ft in range(8)]
        with ExitStack() as ph:
            gp = ph.enter_context(tc.tile_pool(name="otld", bufs=2))
            otp = ph.enter_context(tc.tile_pool(name="ot", bufs=1))
            fps = ph.enter_context(
                tc.tile_pool(name="o2ps", bufs=3, space="PSUM"))
            OT = [otp.tile([128, TOK], BF16, name=f"OT{rc}")
                  for rc in range(4)]
            for rc in range(4):
                g = gp.tile([128, TOK], F32, name="g")
                nc.sync.dma_start(g[:], rs_out[rc * 128:(rc + 1) * 128, :])
                nc.vector.tensor_copy(OT[rc][:], g[:])
            for ft in range(8):
                ps = fps.tile([128, TOK], F32, name="o2")
                for rc in range(4):
                    nc.tensor.matmul(
                        ps[:],
                        ovT_sb[:, rc * D + ft * 128:rc * D + (ft + 1) * 128],
                        OT[rc][:], start=(rc == 0), stop=(rc == 3))
                nc.vector.scalar_tensor_tensor(
                    hT[ft][:], ps[:], csb["ob"][:, ft:ft + 1], hr_sb[ft][:],
                    ALU.add, ALU.add)

        # ---------------- Phase I: LN2 -> z^T (bf16) ----------------
        if phases < 5:
            return nc, None
        zp = top.enter_context(tc.tile_pool(name="zT", bufs=1))
        zT = [zp.tile([128, TOK], BF16, name=f"z{fc}") for fc in range(8)]
        with ExitStack() as ph:
            tmp = ph.enter_context(tc.tile_pool(name="ln2tmp", bufs=6))
            stp = ph.enter_context(tc.tile_pool(name="ln2stat", bufs=1))
            sps = ph.enter_context(
                tc.tile_pool(name="ln2ps", bufs=2, space="PSUM"))
            sum_ps = sps.tile([1, TOK], F32, name="sum")
            sq_ps = sps.tile([1, TOK], F32, name="sq")
            for fc in range(8):
                hbf = tmp.tile([128, TOK], BF16, name="hbf")
                nc.vector.tensor_copy(hbf[:], hT[fc][:])
                sq = tmp.tile([128, TOK], BF16, name="sq2")
                nc.vector.tensor_mul(sq[:], hbf[:], hbf[:])
                nc.tensor.matmul(sum_ps[:], ones_t[:], hbf[:],
                                 start=(fc == 0), stop=(fc == 7))
                nc.tensor.matmul(sq_ps[:], ones_t[:], sq[:],
                                 start=(fc == 0), stop=(fc == 7))
            mu_c = stp.tile([1, TOK], F32, name="mu2c")
            w1 = stp.tile([1, TOK], F32, name="w12")
            t2 = stp.tile([1, TOK], F32, name="t22")
            nc.scalar.mul(mu_c[:], sum_ps[:], 1.0 / D)
            nc.scalar.mul(w1[:], sq_ps[:], 1.0 / D)
            nc.vector.tensor_mul(t2[:], mu_c[:], mu_c[:])
            nc.vector.tensor_sub(w1[:], w1[:], t2[:])
            nc.scalar.activation(t2[:], w1[:], AF.Sqrt, bias=eps1[:])
            nc.vector.reciprocal(w1[:], t2[:])
            mu_b = stp.tile([128, TOK], F32, name="mub2")
            r_b = stp.tile([128, TOK], F32, name="rb2")
            nc.gpsimd.partition_broadcast(mu_b[:], mu_c[:])
            nc.gpsimd.partition_broadcast(r_b[:], w1[:])
            for fc in range(8):
                t1 = tmp.tile([128, TOK], F32, name="ap2")
                nc.vector.tensor_sub(t1[:], hT[fc][:], mu_b[:])
                nc.vector.tensor_mul(t1[:], t1[:], r_b[:])
                nc.vector.tensor_scalar(zT[fc][:], t1[:],
                                        csb["ln2g"][:, fc:fc + 1],
                                        csb["ln2b"][:, fc:fc + 1],
                                        ALU.mult, ALU.add)

        # ---------------- Phase J..M: FFN ----------------
        if phases < 6:
            return nc, None
        with ExitStack() as ph:
            fps = ph.enter_context(
                tc.tile_pool(name="ffnps", bufs=4, space="PSUM"))
            ap1 = ph.enter_context(tc.tile_pool(name="a1", bufs=1))
            h1p = ph.enter_context(tc.tile_pool(name="h1", bufs=1))
            ap2 = ph.enter_context(tc.tile_pool(name="a2", bufs=1))
            outp = ph.enter_context(tc.tile_pool(name="outT", bufs=2))
            a1 = [ap1.tile([128, TOK], BF16, name=f"a1{rt}")
                  for rt in range(4)]
            for rt in range(4):
                ps = fps.tile([128, TOK], F32, name="fpsb")
                for fc in range(8):
                    nc.tensor.matmul(
                        ps[:],
                        f1u_sb[:, fc * RFC + rt * 128:
                               fc * RFC + (rt + 1) * 128],
                        zT[fc][:], start=(fc == 0), stop=(fc == 7))
                nc.vector.tensor_copy(a1[rt][:], ps[:])
            h1 = [h1p.tile([128, TOK], BF16, name=f"h1{it}")
                  for it in range(32)]
            for it in range(32):
                ps = fps.tile([128, TOK], F32, name="fpsb")
                for rc in range(4):
                    nc.tensor.matmul(
                        ps[:],
                        f1v_sb[:, rc * I + it * 128:rc * I + (it + 1) * 128],
                        a1[rc][:], start=(rc == 0), stop=(rc == 3))
                nc.scalar.activation(h1[it][:], ps[:], AF.Gelu,
                                     bias=f1b_sb[:, it:it + 1])
            a2 = [ap2.tile([128, TOK], BF16, name=f"a2{rt}")
                  for rt in range(4)]
            for rt in range(4):
                ps = fps.tile([128, TOK], F32, name="fpsb")
                for ic in range(32):
                    nc.tensor.matmul(
                        ps[:],
                        f2u_sb[:, ic * RFC + rt * 128:
                               ic * RFC + (rt + 1) * 128],
                        h1[ic][:], start=(ic == 0), stop=(ic == 31))
                nc.vector.tensor_copy(a2[rt][:], ps[:])
            for ft in range(8):
                ps = fps.tile([128, TOK], F32, name="fpsb")
                for rc in range(4):
                    nc.tensor.matmul(
                        ps[:],
                        f2v_sb[:, rc * D + ft * 128:rc * D + (ft + 1) * 128],
                        a2[rc][:], start=(rc == 0), stop=(rc == 3))
                ot = outp.tile([128, TOK], F32, name="o")
                nc.vector.scalar_tensor_tensor(
                    ot[:], ps[:], csb["f2b"][:, ft:ft + 1], hT[ft][:],
                    ALU.add, ALU.add)
                nc.sync.dma_start(out_t[ft * 128:(ft + 1) * 128, :], ot[:])

    nc.compile()
    return nc


def _prep_inputs(inputs):
    bf = ml_dtypes.bfloat16
    hs = np.asarray(inputs["hidden_states"], np.float32)

    def pt8(v):  # [1024] -> [128, 8]
        return np.ascontiguousarray(
            np.asarray(v, np.float32).reshape(8, 128).T)

    shared = {
        "ln1g": pt8(inputs["ln1_g"]), "ln1b": pt8(inputs["ln1_b"]),
        "ln2g": pt8(inputs["ln2_g"]), "ln2b": pt8(inputs["ln2_b"]),
        "ob": pt8(inputs["out_b"]), "f2b": pt8(inputs["fc2_b"]),
        "f1b": np.ascontiguousarray(
            np.asarray(inputs["fc1_b"], np.float32).reshape(32, 128).T),
        "tri": np.triu(np.ones((128, 128))).astype(bf),
        "ones1": np.ones((128, 1), bf),
        "ovT": np.asarray(inputs["out_V"]).astype(bf),
        "f1u": np.asarray(inputs["fc1_U"]).astype(bf),
        "f1v": np.asarray(inputs["fc1_V"]).astype(bf),
        "f2u": np.asarray(inputs["fc2_U"]).astype(bf),
        "f2v": np.asarray(inputs["fc2_V"]).astype(bf),
    }
    qU, kU, vU = (np.asarray(inputs[k], np.float32)
                  for k in ("q_U", "k_U", "v_U"))
    qV, kV, vV = (np.asarray(inputs[k], np.float32)
                  for k in ("q_V", "k_V", "v_V"))
    qb, kb, vb = (np.asarray(inputs[k], np.float32)
                  for k in ("q_b", "k_b", "v_b"))
    oU = np.asarray(inputs["out_U"], np.float32)

    in_maps = []
    for c in range(NCORE):
        b, g = c // 4, c % 4
        hsel = slice(4 * g, 4 * g + 4)
        m = dict(shared)
        m["hbT"] = np.ascontiguousarray(hs[b].T).astype(bf)
        m["hrT"] = np.ascontiguousarray(
            hs[b, 512 * g:512 * (g + 1), :].T)
        g1 = np.asarray(inputs["ln1_g"], np.float32)
        b1 = np.asarray(inputs["ln1_b"], np.float32)
        for nm, U in (("uq", qU), ("uk", kU), ("uv", vU)):
            ue = U[:, hsel, :].reshape(D, HPC * R) * g1[:, None]
            m[nm] = np.ascontiguousarray(ue).astype(bf)
            m[nm + "c"] = np.ascontiguousarray(-ue.sum(0)[:, None])
            m[nm + "b"] = np.ascontiguousarray(
                (b1 @ U[:, hsel, :].reshape(D, HPC * R))[:, None])
        for nm, V in (("vq", qV), ("vk", kV), ("vv", vV)):
            m[nm] = np.ascontiguousarray(
                V[hsel].transpose(1, 0, 2).reshape(R, HPC * DH)).astype(bf)
        for nm, bb in (("bq", qb), ("bk", kb), ("bv", vb)):
            m[nm] = np.ascontiguousarray(bb[hsel].T)
        m["ouT"] = np.ascontiguousarray(
            oU[256 * g:256 * (g + 1), :]).astype(bf)
        in_maps.append(m)
    return in_maps


def kernel(trace=False, tmpdir=None, **inputs):
    if "nc" not in _cache:
        _cache["nc"] = _build_program()
    nc = _cache["nc"]
    in_maps = _prep_inputs(inputs)
    res = bass_utils.run_bass_kernel_spmd(
        nc, in_maps, core_ids=list(range(NCORE)), trace=trace,
        tmpdir=tmpdir)
    out = np.zeros((B, S, D), np.float32)
    for c in range(NCORE):
        b, g = c // 4, c % 4
        out[b, 512 * g:512 * (g + 1), :] = res.results[c]["out_t"].T
    if trace:
        return out, res
    return out

